# revision 1
# baseline (speedup 1.0000x reference)
"""Multi-head causal attention with RoPE on 8 TRN2 NeuronCores.

Sharding: 8 cores = 2 batches x 4 head-groups (4 heads each).
Per-core Bass kernel computes the group-partial output transposed;
host sums group partials and transposes back.

All matmuls run in float32r (fp32 storage, ~2e-4 relative error,
~bf16 throughput at moving-dim 512).
"""

import numpy as np

import concourse.bass as bass  # noqa: F401
import concourse.tile as tile
from concourse import bacc, mybir

B, S, D, H, HD = 2, 2048, 2048, 16, 128
NCORES = 8
G = 4            # head groups
GH = 4           # heads per group
GD = GH * HD     # 512 dims per group
P = 128
SQ_U = S // 512  # 4 query slices
SK_T = S // P    # 16 key tiles

_f32 = mybir.dt.float32
_f32r = mybir.dt.float32r

_cache = {}


def _build(causal: bool, reps: int = 1, phases: str = "paw",
           xu_bufs: int = 3, s_bufs: int = 3, o_bufs: int = 1, depth: int = 3,
           app_bufs: int = 6, akv_bufs: int = 2, aq_bufs: int = 2, pt_bufs: int = 3,
           pq_bufs: int = 5, pv_bufs: int = 3, po_bufs: int = 4):
    nc = bacc.Bacc("TRN2", target_bir_lowering=False, debug=False)
    xT = nc.dram_tensor("xT", [D, S], _f32r, kind="ExternalInput").ap()
    wq = nc.dram_tensor("wq", [D, GD], _f32r, kind="ExternalInput").ap()
    wk = nc.dram_tensor("wk", [D, GD], _f32r, kind="ExternalInput").ap()
    wv = nc.dram_tensor("wv", [D, GD], _f32r, kind="ExternalInput").ap()
    wo = nc.dram_tensor("wo", [GD, D], _f32r, kind="ExternalInput").ap()
    cs = nc.dram_tensor("cs", [P, S], _f32, kind="ExternalInput").ap()
    ss = nc.dram_tensor("ss", [P, S], _f32, kind="ExternalInput").ap()
    ones = nc.dram_tensor("ones", [P, P], _f32r, kind="ExternalInput").ap()
    if causal:
        maskd = nc.dram_tensor("maskd", [P, SK_T * 512], _f32, kind="ExternalInput").ap()
    else:
        maskf = nc.dram_tensor("maskf", [S, S], _f32, kind="ExternalInput").ap()
    outT = nc.dram_tensor("outT", [D, S], _f32, kind="ExternalOutput").ap()

    with tile.TileContext(nc) as tc:
      for _rep in range(reps):
        with (
            tc.tile_pool(name="persist", bufs=1) as persist,
            tc.tile_pool(name="dram", bufs=1, space="DRAM") as dpool,
        ):
            qtd = dpool.tile([P, GH * S], _f32r, tag="qtd")   # Q^T per head [hd, S]
            ktd = dpool.tile([P, GH * S], _f32r, tag="ktd")
            vd = dpool.tile([P, SK_T * GD], _f32r, tag="vd")  # V natural
            ones_s = persist.tile([P, P], _f32r, tag="ones")
            nc.sync.dma_start(ones_s[:], ones[:])

            # ---- Phase P: Q^T/K^T/V projections + RoPE -> DRAM scratch
            if "p" in phases:
              with (
                tc.tile_pool(name="pw", bufs=1) as pw,
                tc.tile_pool(name="px", bufs=xu_bufs) as px,
                tc.tile_pool(name="pt", bufs=pt_bufs) as ptp,
                tc.tile_pool(name="po", bufs=po_bufs) as po,
                tc.tile_pool(name="ps1", bufs=pq_bufs, space="PSUM") as ps1,
                tc.tile_pool(name="ps2", bufs=pv_bufs, space="PSUM") as ps2,
              ):
                wq_s = pw.tile([P, SK_T * GD], _f32r, tag="wq")
                wk_s = pw.tile([P, SK_T * GD], _f32r, tag="wk")
                wv_s = pw.tile([P, SK_T * GD], _f32r, tag="wv")
                cs_s = pw.tile([P, S], _f32, tag="cs")
                ss_s = pw.tile([P, S], _f32, tag="ss")
                def _load_xu(u):
                    xh = []
                    for half in range(2):
                        xu = px.tile([P, 8 * 512], _f32r, tag="xu")
                        for j in range(8):
                            t = half * 8 + j
                            nc.sync.dma_start(
                                xu[:, j * 512:(j + 1) * 512],
                                xT[t * P:(t + 1) * P, u * 512:(u + 1) * 512])
                        xh.append(xu)
                    return xh

                nc.sync.dma_start(cs_s[:], cs[:])
                nc.sync.dma_start(ss_s[:], ss[:])
                xh0 = _load_xu(0)
                for t in range(SK_T):
                    nc.sync.dma_start(wq_s[:, t * GD:(t + 1) * GD], wq[t * P:(t + 1) * P, :])
                for t in range(SK_T):
                    nc.sync.dma_start(wk_s[:, t * GD:(t + 1) * GD], wk[t * P:(t + 1) * P, :])
                    nc.sync.dma_start(wv_s[:, t * GD:(t + 1) * GD], wv[t * P:(t + 1) * P, :])
                for u in range(SQ_U):
                    xh = xh0 if u == 0 else _load_xu(u)
                    su = slice(u * 512, (u + 1) * 512)
                    # Q^T and K^T (transposed) with fused RoPE
                    for (w_s, dst) in ((wq_s, qtd), (wk_s, ktd)):
                        for dt in range(GH):
                            pq = ps1.tile([P, 512], _f32, tag="pq")
                            for t in range(SK_T):
                                nc.tensor.matmul(
                                    pq[:],
                                    w_s[:, t * GD + dt * P: t * GD + dt * P + P],
                                    xh[t // 8][:, (t % 8) * 512:(t % 8 + 1) * 512],
                                    start=(t == 0), stop=(t == SK_T - 1))
                            t1 = ptp.tile([P, 512], _f32, tag="t1")
                            t2 = ptp.tile([P, 512], _f32, tag="t2")
                            nc.vector.tensor_mul(t1[:], pq[:], cs_s[:, su])
                            nc.vector.tensor_mul(t2[0:64, :], pq[64:P, :], ss_s[0:64, su])
                            nc.vector.tensor_mul(t2[64:P, :], pq[0:64, :], ss_s[64:P, su])
                            ro = po.tile([P, 512], _f32r, tag="ro")
                            nc.vector.tensor_add(ro[:], t1[:], t2[:])
                            nc.sync.dma_start(
                                dst[:, dt * S + u * 512: dt * S + (u + 1) * 512], ro[:])
                    # V (natural layout)
                    for st in range(4):
                        g = 4 * u + st
                        pv = ps2.tile([P, GD], _f32, tag="pv")
                        for t in range(SK_T):
                            nc.tensor.matmul(
                                pv[:],
                                xh[t // 8][:, (t % 8) * 512 + st * P: (t % 8) * 512 + st * P + P],
                                wv_s[:, t * GD:(t + 1) * GD],
                                start=(t == 0), stop=(t == SK_T - 1))
                        vo = po.tile([P, GD], _f32r, tag="vo")
                        nc.scalar.copy(vo[:], pv[:])
                        nc.sync.dma_start(vd[:, g * GD:(g + 1) * GD], vo[:])

            # ---- Phases A (attention) + W (output projection)
            if "a" in phases or "w" in phases:
              with (
                tc.tile_pool(name="amask", bufs=1) as amask_p,
                tc.tile_pool(name="akv", bufs=1) as akv,
                tc.tile_pool(name="aq", bufs=aq_bufs) as aq,
                tc.tile_pool(name="app", bufs=app_bufs) as app,
                tc.tile_pool(name="ar", bufs=2) as ar,
                tc.tile_pool(name="aw", bufs=1) as aw,
                tc.tile_pool(name="wst", bufs=3) as wst,
                tc.tile_pool(name="ps3", bufs=s_bufs, space="PSUM") as ps3,
                tc.tile_pool(name="ps3b", bufs=2, space="PSUM") as ps3b,
                tc.tile_pool(name="ps4", bufs=o_bufs, space="PSUM") as ps4,
              ):
                wo_s = aw.tile([P, GH * S], _f32r, tag="wo")
                aot = aw.tile([P, GH * S], _f32r, tag="aot")
                for dt in range(GH):
                    nc.sync.dma_start(wo_s[:, dt * S:(dt + 1) * S], wo[dt * P:(dt + 1) * P, :])
                if causal:
                    md_s = amask_p.tile([P, SK_T * 512], _f32, tag="md")
                    nc.sync.dma_start(md_s[:], maskd[:])
                kt_all = akv.tile([P, GH * S], _f32r, tag="kt")
                v_all = akv.tile([P, GH * S], _f32r, tag="vh")
                for u in range(SQ_U if "a" in phases else 0):
                    # causal: K/V slices arrive cumulatively (prefix attention);
                    # generic: every chain reads all slices, so load them all at u=0
                    for lu in ([u] if causal else (range(SQ_U) if u == 0 else [])):
                        for h in range(GH):
                            nc.sync.dma_start(
                                kt_all[:, h * S + lu * 512: h * S + (lu + 1) * 512],
                                ktd[:, h * S + lu * 512: h * S + (lu + 1) * 512])
                            for st in range(4):
                                t = 4 * lu + st
                                nc.sync.dma_start(
                                    v_all[:, h * S + t * P: h * S + (t + 1) * P],
                                    vd[:, t * GD + h * P: t * GD + (h + 1) * P])
                    for h in range(GH):
                        kt_h = kt_all[:, h * S: (h + 1) * S]
                        v_h = v_all[:, h * S: (h + 1) * S]
                        qu = aq.tile([P, 512], _f32r, tag="qu")
                        nc.sync.dma_start(
                            qu[:], qtd[:, h * S + u * 512: h * S + (u + 1) * 512])
                        if not causal:
                            mu = amask_p.tile([P, SK_T * 512], _f32, tag="mu")
                            for t in range(SK_T):
                                nc.sync.dma_start(
                                    mu[:, t * 512:(t + 1) * 512],
                                    maskf[t * P:(t + 1) * P, u * 512:(u + 1) * 512])
                        n_sk = 4 * (u + 1) if causal else SK_T
                        psd = ps3b.tile([P, 512], _f32, tag="d")
                        psa = ps3b.tile([P, 512], _f32, tag="a")
                        pts = [None] * n_sk

                        def _consume(t):
                            nc.tensor.matmul(psd[:], ones_s[:], pts[t][:],
                                             start=(t == 0), stop=(t == n_sk - 1))
                            nc.tensor.matmul(psa[:], v_h[:, t * P:(t + 1) * P], pts[t][:],
                                             start=(t == 0), stop=(t == n_sk - 1))

                        for t in range(n_sk):
                            pss = ps3.tile([P, 512], _f32, tag="s")
                            nc.tensor.matmul(pss[:], kt_h[:, t * P:(t + 1) * P], qu[:],
                                             start=True, stop=True)
                            if causal:
                                if t >= 4 * u:
                                    nc.vector.tensor_add(
                                        pss[:], pss[:], md_s[:, t * 512:(t + 1) * 512])
                            else:
                                nc.vector.tensor_add(
                                    pss[:], pss[:], mu[:, t * 512:(t + 1) * 512])
                            pt = app.tile([P, 512], _f32r, tag="p")
                            nc.scalar.activation(pt[:], pss[:],
                                                 mybir.ActivationFunctionType.Exp)
                            pts[t] = pt
                            if t >= depth:
                                _consume(t - depth)
                        for t in range(max(0, n_sk - depth), n_sk):
                            _consume(t)
                        rec = ar.tile([P, 512], _f32, tag="rec")
                        nc.vector.reciprocal(rec[:], psd[:])
                        nc.vector.tensor_mul(
                            aot[:, h * S + u * 512: h * S + (u + 1) * 512],
                            psa[:], rec[:])
                    # ---- W(u): streamed output projection for this slice
                    for ot in range(SK_T if "w" in phases else 0):
                        po2 = ps4.tile([P, 512], _f32, tag="o")
                        for dt in range(GH):
                            nc.tensor.matmul(
                                po2[:],
                                wo_s[:, dt * S + ot * P: dt * S + (ot + 1) * P],
                                aot[:, dt * S + u * 512: dt * S + (u + 1) * 512],
                                start=(dt == 0), stop=(dt == GH - 1))
                        so = wst.tile([P, 512], _f32, tag="so")
                        nc.scalar.copy(so[:], po2[:])
                        nc.sync.dma_start(
                            outT[ot * P:(ot + 1) * P, u * 512:(u + 1) * 512], so[:])
    nc.compile()
    return nc


class _Runner:
    """Persistent PJRT executable for one compiled Bass module (SPMD over 8 cores)."""

    def __init__(self, nc, n_cores):
        import jax
        from jax.sharding import Mesh, PartitionSpec
        from jax.experimental.shard_map import shard_map
        from concourse.bass2jax import (
            _bass_exec_p, install_neuronx_cc_hook, partition_id_tensor)

        install_neuronx_cc_hook()
        self.jax = jax
        self.n_cores = n_cores
        partition_name = nc.partition_id_tensor.name if nc.partition_id_tensor else None
        in_names, out_names, out_avals = [], [], []
        for alloc in nc.m.functions[0].allocations:
            if not isinstance(alloc, mybir.MemoryLocationSet):
                continue
            name = alloc.memorylocations[0].name
            if alloc.kind == "ExternalInput":
                if name != partition_name:
                    in_names.append(name)
            elif alloc.kind == "ExternalOutput":
                out_names.append(name)
                out_avals.append(jax.core.ShapedArray(
                    tuple(alloc.tensor_shape), mybir.dt.np(alloc.dtype)))
        self.in_names, self.out_names, self.out_avals = in_names, out_names, out_avals
        n_params, n_outs = len(in_names), len(out_avals)
        all_in = list(in_names) + list(out_names)
        if partition_name is not None:
            all_in.append(partition_name)

        def _body(*args):
            operands = list(args)
            if partition_name is not None:
                operands.append(partition_id_tensor())
            return tuple(_bass_exec_p.bind(
                *operands,
                out_avals=tuple(out_avals), in_names=tuple(all_in),
                out_names=tuple(out_names), lowering_input_output_aliases=(),
                sim_require_finite=True, sim_require_nnan=True, nc=nc))

        devices = jax.devices()[:n_cores]
        mesh = Mesh(np.asarray(devices), ("core",))
        self.sharding = jax.sharding.NamedSharding(mesh, PartitionSpec("core"))
        self.fn = jax.jit(
            shard_map(_body, mesh=mesh,
                      in_specs=(PartitionSpec("core"),) * (n_params + n_outs),
                      out_specs=(PartitionSpec("core"),) * n_outs,
                      check_rep=False),
            keep_unused=True)
        self._dev_args = None

    def put_inputs(self, in_maps):
        jax = self.jax
        concat_in = [
            np.concatenate([np.asarray(in_maps[c][n]) for c in range(self.n_cores)], axis=0)
            for n in self.in_names]
        concat_zeros = [
            np.zeros((self.n_cores * a.shape[0], *a.shape[1:]), a.dtype)
            for a in self.out_avals]
        self._dev_args = [
            jax.device_put(v, self.sharding) for v in concat_in + concat_zeros]
        for a in self._dev_args:
            a.block_until_ready()

    def execute(self):
        return self.fn(*self._dev_args)

    def run(self, in_maps):
        last_err = None
        for attempt in range(3):
            try:
                self.put_inputs(in_maps)
                outs = self.execute()
                self.jax.block_until_ready(outs)
                return [
                    {n: np.asarray(outs[i]).reshape(
                        self.n_cores, *self.out_avals[i].shape)[c]
                     for i, n in enumerate(self.out_names)}
                    for c in range(self.n_cores)]
            except Exception as e:  # transient NRT faults: retry
                last_err = e
                import time
                time.sleep(2.0 * (attempt + 1))
        raise last_err


def _get_runner(causal: bool):
    if causal not in _cache:
        _cache[causal] = _Runner(_build(causal), NCORES)
    return _cache[causal]


def _host_prep(x, mask, Wq, Wk, Wv, Wo, causal):
    scale = np.float32(1.0) / np.sqrt(np.float32(HD))
    perm = np.concatenate(
        [np.concatenate([np.arange(0, HD, 2), np.arange(1, HD, 2)]) + HD * hh
         for hh in range(GH)])
    inv = (np.float32(1.0) / np.power(
        np.float32(10000.0),
        np.arange(0, HD, 2).astype(np.float32) / np.float32(HD))).astype(np.float32)
    ang = np.arange(S, dtype=np.float32)[:, None] * inv[None, :]
    cos_t = np.cos(ang).T.astype(np.float32)
    sin_t = np.sin(ang).T.astype(np.float32)
    cs_host = np.ascontiguousarray(np.concatenate([cos_t, cos_t], axis=0))
    ss_host = np.ascontiguousarray(np.concatenate([-sin_t, sin_t], axis=0))
    ones_host = np.ones((P, P), np.float32)
    maskT = np.ascontiguousarray(mask.T)
    if causal:
        md = np.empty((P, SK_T * 512), np.float32)
        for t in range(SK_T):
            u = t // 4
            md[:, t * 512:(t + 1) * 512] = maskT[t * P:(t + 1) * P, u * 512:(u + 1) * 512]
    xTs = [np.ascontiguousarray(x[b].T) for b in range(B)]
    in_maps = []
    for c in range(NCORES):
        b, g = c // G, c % G
        rows = slice(g * GD, (g + 1) * GD)
        m = {
            "xT": xTs[b],
            "wq": np.ascontiguousarray(Wq[rows].T[:, perm] * scale),
            "wk": np.ascontiguousarray(Wk[rows].T[:, perm]),
            "wv": np.ascontiguousarray(Wv[rows].T),
            "wo": np.ascontiguousarray(Wo[:, rows].T),
            "cs": cs_host,
            "ss": ss_host,
            "ones": ones_host,
        }
        if causal:
            m["maskd"] = md
        else:
            m["maskf"] = maskT
        in_maps.append(m)
    return in_maps


def kernel(x, mask, Wq, Wk, Wv, Wo):
    x = np.asarray(x, dtype=np.float32)
    mask = np.asarray(mask, dtype=np.float32)
    Wq = np.asarray(Wq, dtype=np.float32)
    Wk = np.asarray(Wk, dtype=np.float32)
    Wv = np.asarray(Wv, dtype=np.float32)
    Wo = np.asarray(Wo, dtype=np.float32)
    expected_mask = np.triu(np.full((S, S), -1e9, dtype=np.float32), k=1)
    causal = bool(np.array_equal(mask, expected_mask))
    runner = _get_runner(causal)
    in_maps = _host_prep(x, mask, Wq, Wk, Wv, Wo, causal)
    results = runner.run(in_maps)
    out = np.empty((B, S, D), np.float32)
    for b in range(B):
        acc = results[b * G]["outT"].copy()
        for g in range(1, G):
            acc += results[b * G + g]["outT"]
        out[b] = acc.T
    return out



# revision 4
# speedup vs baseline: 1.3236x; 1.3236x over previous
"""Multi-head causal attention with RoPE on 8 TRN2 NeuronCores.

Sharding: 8 cores = 2 batches x 4 head-groups (4 heads each).
Per-core Bass kernel computes the group-partial output transposed;
host sums group partials and transposes back.

All matmul operands are bf16 (same PE throughput as fp32r, half the
SBUF/DMA footprint), accumulation in fp32 PSUM. Q^T/K^T/V stay
resident in SBUF (no DRAM scratch round-trip). Causal masking uses
a single triangular 128x128 tile: within each diagonal 128x512 score
tile only one 128-col block is partially masked; fully-masked columns
are skipped by slicing the matmul moving dim.
"""

import numpy as np
import ml_dtypes

import concourse.bass as bass  # noqa: F401
import concourse.tile as tile
from concourse import bacc, mybir

B, S, D, H, HD = 2, 2048, 2048, 16, 128
NCORES = 8
G = 4            # head groups
GH = 4           # heads per group
GD = GH * HD     # 512 dims per group
P = 128
NU = S // 512    # 4 query slices
NT = S // P      # 16 key tiles

_f32 = mybir.dt.float32
_bf16 = mybir.dt.bfloat16
_bf = ml_dtypes.bfloat16

_cache = {}


def _build(causal: bool, reps: int = 1, depth: int = 4):
    nc = bacc.Bacc("TRN2", target_bir_lowering=False, debug=False)
    xT = nc.dram_tensor("xT", [D, S], _bf16, kind="ExternalInput").ap()
    wq = nc.dram_tensor("wq", [D, GD], _bf16, kind="ExternalInput").ap()
    wk = nc.dram_tensor("wk", [D, GD], _bf16, kind="ExternalInput").ap()
    wv = nc.dram_tensor("wv", [D, GD], _bf16, kind="ExternalInput").ap()
    wo = nc.dram_tensor("wo", [GD, D], _bf16, kind="ExternalInput").ap()
    cs = nc.dram_tensor("cs", [P, S], _f32, kind="ExternalInput").ap()
    ss = nc.dram_tensor("ss", [P, S], _f32, kind="ExternalInput").ap()
    ones = nc.dram_tensor("ones", [P, P], _bf16, kind="ExternalInput").ap()
    if causal:
        tri = nc.dram_tensor("tri", [P, P], _f32, kind="ExternalInput").ap()
    else:
        maskf = nc.dram_tensor("maskf", [S, S], _bf16, kind="ExternalInput").ap()
    outT = nc.dram_tensor("outT", [D, S], _bf16, kind="ExternalOutput").ap()

    Exp = mybir.ActivationFunctionType.Exp

    with tile.TileContext(nc) as tc:
      with (
          tc.tile_pool(name="pw", bufs=1) as pw,
          tc.tile_pool(name="pcon", bufs=1) as pcon,
          tc.tile_pool(name="px", bufs=2 if causal else 1) as px,
          tc.tile_pool(name="pqkv", bufs=1) as pqkv,
          tc.tile_pool(name="paot", bufs=2) as paot,
          tc.tile_pool(name="ptmp", bufs=3) as ptmp,
          tc.tile_pool(name="ppt", bufs=6) as ppt,
          tc.tile_pool(name="prec", bufs=2) as prec,
          tc.tile_pool(name="pso", bufs=4) as pso,
          tc.tile_pool(name="pmu", bufs=1) as pmu,
          tc.tile_pool(name="psAB", bufs=2, space="PSUM") as psAB,
          tc.tile_pool(name="psB", bufs=4, space="PSUM") as psB,
      ):
        for _rep in range(reps):
            # ---- per-rep input tiles (pools persist; tags rotate buffers)
            wq_s = pw.tile([P, NT * GD], _bf16, tag="wq")
            wk_s = pw.tile([P, NT * GD], _bf16, tag="wk")
            wv_s = pw.tile([P, NT * GD], _bf16, tag="wv")
            wo_s = pw.tile([P, GH * D], _bf16, tag="wo")
            cs_s = pcon.tile([P, S], _f32, tag="cs")
            ss_s = pcon.tile([P, S], _f32, tag="ss")
            ones_s = pcon.tile([P, P], _bf16, tag="ones")
            if causal:
                tri_s = pcon.tile([P, P], _f32, tag="tri")
                nc.sync.dma_start(tri_s[:], tri[:])
            qt_s = pqkv.tile([P, GH * S], _bf16, tag="qt")
            kt_s = pqkv.tile([P, GH * S], _bf16, tag="kt")
            v_s = pqkv.tile([P, NT * GD], _bf16, tag="v")

            nc.sync.dma_start(cs_s[:], cs[:])
            nc.sync.dma_start(ss_s[:], ss[:])
            nc.sync.dma_start(ones_s[:], ones[:])
            for t in range(NT):
                nc.sync.dma_start(wq_s[:, t * GD:(t + 1) * GD], wq[t * P:(t + 1) * P, :])
            for t in range(NT):
                nc.sync.dma_start(wk_s[:, t * GD:(t + 1) * GD], wk[t * P:(t + 1) * P, :])
                nc.sync.dma_start(wv_s[:, t * GD:(t + 1) * GD], wv[t * P:(t + 1) * P, :])
            for dt in range(GH):
                nc.sync.dma_start(wo_s[:, dt * D:(dt + 1) * D], wo[dt * P:(dt + 1) * P, :])

            # ---- Phase P: Q^T/K^T (RoPE fused) and V projections -> SBUF
            for u in range(NU):
                su = slice(u * 512, (u + 1) * 512)
                xu = px.tile([P, NT * GD], _bf16, tag="xu")
                for t in range(NT):
                    nc.sync.dma_start(
                        xu[:, t * GD:(t + 1) * GD],
                        xT[t * P:(t + 1) * P, u * 512:(u + 1) * 512])
                for (w_s, dst) in ((wq_s, qt_s), (wk_s, kt_s)):
                    for dt in range(GH):
                        pq = psB.tile([P, 512], _f32, tag="ps")
                        for t in range(NT):
                            nc.tensor.matmul(
                                pq[:],
                                w_s[:, t * GD + dt * P: t * GD + dt * P + P],
                                xu[:, t * GD:(t + 1) * GD],
                                start=(t == 0), stop=(t == NT - 1))
                        t1 = ptmp.tile([P, 512], _f32, tag="t1")
                        t2 = ptmp.tile([P, 512], _f32, tag="t2")
                        nc.vector.tensor_mul(t1[:], pq[:], cs_s[:, su])
                        nc.vector.tensor_mul(t2[0:64, :], pq[64:P, :], ss_s[0:64, su])
                        nc.vector.tensor_mul(t2[64:P, :], pq[0:64, :], ss_s[64:P, su])
                        nc.vector.tensor_add(
                            dst[:, dt * S + u * 512: dt * S + (u + 1) * 512],
                            t1[:], t2[:])
                for st in range(4):
                    g = 4 * u + st
                    pv = psB.tile([P, GD], _f32, tag="ps")
                    for t in range(NT):
                        nc.tensor.matmul(
                            pv[:],
                            xu[:, t * GD + st * P: t * GD + st * P + P],
                            wv_s[:, t * GD:(t + 1) * GD],
                            start=(t == 0), stop=(t == NT - 1))
                    nc.scalar.copy(v_s[:, g * GD:(g + 1) * GD], pv[:])

            # ---- Phases A (attention) + W (output projection), staggered
            def attn(u, h):
                n_sk = 4 * (u + 1) if causal else NT
                if not causal:
                    mu = mus[u]
                psa = psAB.tile([P, 512], _f32, tag="psa")
                psd = psAB.tile([P, 512], _f32, tag="psd")
                pts = [None] * n_sk

                def consume(t):
                    pt, lo = pts[t]
                    nc.tensor.matmul(psd[:, lo:512], ones_s[:],
                                     pt[:, lo:512],
                                     start=(t == 0), stop=(t == n_sk - 1))
                    nc.tensor.matmul(psa[:, lo:512],
                                     v_s[:, t * GD + h * P: t * GD + (h + 1) * P],
                                     pt[:, lo:512],
                                     start=(t == 0), stop=(t == n_sk - 1))

                for t in range(n_sk):
                    d = t - 4 * u
                    lo = P * d if (causal and d > 0) else 0
                    pss = psB.tile([P, 512], _f32, tag="ps")
                    nc.tensor.matmul(
                        pss[:, lo:512],
                        kt_s[:, h * S + t * P: h * S + (t + 1) * P],
                        qt_s[:, h * S + u * 512 + lo: h * S + (u + 1) * 512],
                        start=True, stop=True)
                    if causal:
                        if d >= 0:
                            nc.vector.tensor_add(
                                pss[:, lo:lo + P], pss[:, lo:lo + P], tri_s[:])
                    else:
                        nc.vector.tensor_add(
                            pss[:], pss[:], mu[:, t * 512:(t + 1) * 512])
                    pt = ppt.tile([P, 512], _bf16, tag="pt")
                    nc.scalar.activation(pt[:, lo:512], pss[:, lo:512], Exp)
                    pts[t] = (pt, lo)
                    if t >= depth:
                        consume(t - depth)
                for t in range(max(0, n_sk - depth), n_sk):
                    consume(t)
                rec = prec.tile([P, 512], _f32, tag="rec")
                nc.vector.reciprocal(rec[:], psd[:])
                nc.vector.tensor_mul(
                    aots[u][:, h * 512:(h + 1) * 512], psa[:], rec[:])

            def wproj(u):
                for ot in range(NT):
                    po2 = psB.tile([P, 512], _f32, tag="ps")
                    for dt in range(GH):
                        nc.tensor.matmul(
                            po2[:],
                            wo_s[:, dt * D + ot * P: dt * D + (ot + 1) * P],
                            aots[u][:, dt * 512:(dt + 1) * 512],
                            start=(dt == 0), stop=(dt == GH - 1))
                    so = pso.tile([P, 512], _bf16, tag="so")
                    nc.vector.tensor_copy(so[:], po2[:])
                    nc.sync.dma_start(
                        outT[ot * P:(ot + 1) * P, u * 512:(u + 1) * 512], so[:])

            aots = {}
            mus = {}
            for u in range(NU):
                aots[u] = paot.tile([P, GH * 512], _bf16, tag="aot", name="aot")
                if not causal:
                    mus[u] = pmu.tile([P, NT * 512], _bf16, tag="mu", name="mu")
                    for t in range(NT):
                        nc.sync.dma_start(
                            mus[u][:, t * 512:(t + 1) * 512],
                            maskf[t * P:(t + 1) * P, u * 512:(u + 1) * 512])
                attn(u, 0)
                if u > 0:
                    wproj(u - 1)
                for h in range(1, GH):
                    attn(u, h)
            wproj(NU - 1)
    nc.compile()
    return nc


class _Runner:
    """Persistent PJRT executable for one compiled Bass module (SPMD over 8 cores)."""

    def __init__(self, nc, n_cores):
        import jax
        from jax.sharding import Mesh, PartitionSpec
        from jax.experimental.shard_map import shard_map
        from concourse.bass2jax import (
            _bass_exec_p, install_neuronx_cc_hook, partition_id_tensor)

        install_neuronx_cc_hook()
        self.jax = jax
        self.n_cores = n_cores
        partition_name = nc.partition_id_tensor.name if nc.partition_id_tensor else None
        in_names, out_names, out_avals = [], [], []
        for alloc in nc.m.functions[0].allocations:
            if not isinstance(alloc, mybir.MemoryLocationSet):
                continue
            name = alloc.memorylocations[0].name
            if alloc.kind == "ExternalInput":
                if name != partition_name:
                    in_names.append(name)
            elif alloc.kind == "ExternalOutput":
                out_names.append(name)
                out_avals.append(jax.core.ShapedArray(
                    tuple(alloc.tensor_shape), mybir.dt.np(alloc.dtype)))
        self.in_names, self.out_names, self.out_avals = in_names, out_names, out_avals
        n_params, n_outs = len(in_names), len(out_avals)
        all_in = list(in_names) + list(out_names)
        if partition_name is not None:
            all_in.append(partition_name)

        def _body(*args):
            operands = list(args)
            if partition_name is not None:
                operands.append(partition_id_tensor())
            return tuple(_bass_exec_p.bind(
                *operands,
                out_avals=tuple(out_avals), in_names=tuple(all_in),
                out_names=tuple(out_names), lowering_input_output_aliases=(),
                sim_require_finite=True, sim_require_nnan=True, nc=nc))

        devices = jax.devices()[:n_cores]
        mesh = Mesh(np.asarray(devices), ("core",))
        self.sharding = jax.sharding.NamedSharding(mesh, PartitionSpec("core"))
        self.fn = jax.jit(
            shard_map(_body, mesh=mesh,
                      in_specs=(PartitionSpec("core"),) * (n_params + n_outs),
                      out_specs=(PartitionSpec("core"),) * n_outs,
                      check_rep=False),
            keep_unused=True)
        self._dev_args = None

    def put_inputs(self, in_maps):
        jax = self.jax
        concat_in = [
            np.concatenate([np.asarray(in_maps[c][n]) for c in range(self.n_cores)], axis=0)
            for n in self.in_names]
        concat_zeros = [
            np.zeros((self.n_cores * a.shape[0], *a.shape[1:]), a.dtype)
            for a in self.out_avals]
        self._dev_args = [
            jax.device_put(v, self.sharding) for v in concat_in + concat_zeros]
        for a in self._dev_args:
            a.block_until_ready()

    def execute(self):
        return self.fn(*self._dev_args)

    def run(self, in_maps):
        last_err = None
        for attempt in range(3):
            try:
                self.put_inputs(in_maps)
                outs = self.execute()
                self.jax.block_until_ready(outs)
                return [
                    {n: np.asarray(outs[i]).reshape(
                        self.n_cores, *self.out_avals[i].shape)[c]
                     for i, n in enumerate(self.out_names)}
                    for c in range(self.n_cores)]
            except Exception as e:  # transient NRT faults: retry
                last_err = e
                import time
                time.sleep(2.0 * (attempt + 1))
        raise last_err


def _get_runner(causal: bool):
    if causal not in _cache:
        _cache[causal] = _Runner(_build(causal), NCORES)
    return _cache[causal]


def _host_prep(x, mask, Wq, Wk, Wv, Wo, causal):
    scale = np.float32(1.0) / np.sqrt(np.float32(HD))
    perm = np.concatenate(
        [np.concatenate([np.arange(0, HD, 2), np.arange(1, HD, 2)]) + HD * hh
         for hh in range(GH)])
    inv = (np.float32(1.0) / np.power(
        np.float32(10000.0),
        np.arange(0, HD, 2).astype(np.float32) / np.float32(HD))).astype(np.float32)
    ang = np.arange(S, dtype=np.float32)[:, None] * inv[None, :]
    cos_t = np.cos(ang).T.astype(np.float32)
    sin_t = np.sin(ang).T.astype(np.float32)
    cs_host = np.ascontiguousarray(np.concatenate([cos_t, cos_t], axis=0))
    ss_host = np.ascontiguousarray(np.concatenate([-sin_t, sin_t], axis=0))
    ones_host = np.ones((P, P), _bf)
    if causal:
        # tri[r, j] = -1e9 where key r (within block) > query j (within block)
        tri_host = np.tril(np.full((P, P), np.float32(-1e9)), k=-1).astype(np.float32)
    else:
        maskT = np.ascontiguousarray(mask.T).astype(_bf)
    xTs = [np.ascontiguousarray(x[b].T).astype(_bf) for b in range(B)]
    in_maps = []
    for c in range(NCORES):
        b, g = c // G, c % G
        rows = slice(g * GD, (g + 1) * GD)
        m = {
            "xT": xTs[b],
            "wq": np.ascontiguousarray(Wq[rows].T[:, perm] * scale).astype(_bf),
            "wk": np.ascontiguousarray(Wk[rows].T[:, perm]).astype(_bf),
            "wv": np.ascontiguousarray(Wv[rows].T).astype(_bf),
            "wo": np.ascontiguousarray(Wo[:, rows].T).astype(_bf),
            "cs": cs_host,
            "ss": ss_host,
            "ones": ones_host,
        }
        if causal:
            m["tri"] = tri_host
        else:
            m["maskf"] = maskT
        in_maps.append(m)
    return in_maps


def kernel(x, mask, Wq, Wk, Wv, Wo):
    x = np.asarray(x, dtype=np.float32)
    mask = np.asarray(mask, dtype=np.float32)
    Wq = np.asarray(Wq, dtype=np.float32)
    Wk = np.asarray(Wk, dtype=np.float32)
    Wv = np.asarray(Wv, dtype=np.float32)
    Wo = np.asarray(Wo, dtype=np.float32)
    expected_mask = np.triu(np.full((S, S), -1e9, dtype=np.float32), k=1)
    causal = bool(np.array_equal(mask, expected_mask))
    runner = _get_runner(causal)
    in_maps = _host_prep(x, mask, Wq, Wk, Wv, Wo, causal)
    results = runner.run(in_maps)
    out = np.empty((B, S, D), np.float32)
    for b in range(B):
        acc = results[b * G]["outT"].astype(np.float32)
        for g in range(1, G):
            acc += results[b * G + g]["outT"].astype(np.float32)
        out[b] = acc.T
    return out


# revision 14
# speedup vs baseline: 1.3574x; 1.0256x over previous
"""Multi-head causal attention with RoPE on 8 TRN2 NeuronCores.

Sharding: 8 cores = 2 batches x 4 head-groups (4 heads each).
Per-core Bass kernel computes the group-partial output transposed;
host sums group partials and transposes back.

All matmul operands are bf16 (same PE throughput as fp32r, half the
SBUF/DMA footprint), accumulation in fp32 PSUM. Q^T/K^T/V stay
resident in SBUF (no DRAM scratch round-trip). Causal masking uses
a single triangular 128x128 tile: within each diagonal 128x512 score
tile only one 128-col block is partially masked; fully-masked columns
are skipped by slicing the matmul moving dim.
"""

import numpy as np
import ml_dtypes

import concourse.bass as bass  # noqa: F401
import concourse.tile as tile
from concourse import bacc, mybir

B, S, D, H, HD = 2, 2048, 2048, 16, 128
NCORES = 8
G = 4            # head groups
GH = 4           # heads per group
GD = GH * HD     # 512 dims per group
P = 128
NU = S // 512    # 4 query slices
NT = S // P      # 16 key tiles

_f32 = mybir.dt.float32
_bf16 = mybir.dt.bfloat16
_bf = ml_dtypes.bfloat16

_cache = {}


def _build(causal: bool, reps: int = 1, depth: int = 5):
    nc = bacc.Bacc("TRN2", target_bir_lowering=False, debug=False)
    xT = nc.dram_tensor("xT", [D, S], _bf16, kind="ExternalInput").ap()
    wq = nc.dram_tensor("wq", [D, GD], _bf16, kind="ExternalInput").ap()
    wk = nc.dram_tensor("wk", [D, GD], _bf16, kind="ExternalInput").ap()
    wv = nc.dram_tensor("wv", [D, GD], _bf16, kind="ExternalInput").ap()
    wo = nc.dram_tensor("wo", [GD, D], _bf16, kind="ExternalInput").ap()
    cs = nc.dram_tensor("cs", [P, S], _f32, kind="ExternalInput").ap()
    ss = nc.dram_tensor("ss", [P, S], _f32, kind="ExternalInput").ap()
    ones = nc.dram_tensor("ones", [P, P], _bf16, kind="ExternalInput").ap()
    if causal:
        tri = nc.dram_tensor("tri", [P, P], _bf16, kind="ExternalInput").ap()
    else:
        maskf = nc.dram_tensor("maskf", [S, S], _bf16, kind="ExternalInput").ap()
    outT = nc.dram_tensor("outT", [D, S], _bf16, kind="ExternalOutput").ap()

    Exp = mybir.ActivationFunctionType.Exp

    with tile.TileContext(nc) as tc:
      with (
          tc.tile_pool(name="pw", bufs=1) as pw,
          tc.tile_pool(name="pcon", bufs=1) as pcon,
          tc.tile_pool(name="px", bufs=2 if causal else 1) as px,
          tc.tile_pool(name="pqkv", bufs=1) as pqkv,
          tc.tile_pool(name="paot", bufs=2) as paot,
          tc.tile_pool(name="ptmp", bufs=2) as ptmp,
          tc.tile_pool(name="ppt", bufs=8) as ppt,
          tc.tile_pool(name="pds", bufs=6) as pds,
          tc.tile_pool(name="prec", bufs=2) as prec,
          tc.tile_pool(name="pso", bufs=8) as pso,
          tc.tile_pool(name="pmu", bufs=1) as pmu,
          tc.tile_pool(name="psAB", bufs=2, space="PSUM") as psAB,
          tc.tile_pool(name="psB", bufs=4, space="PSUM") as psB,
      ):
        for _rep in range(reps):
            # ---- per-rep input tiles (pools persist; tags rotate buffers)
            wq_s = pw.tile([P, NT * GD], _bf16, tag="wq")
            wk_s = pw.tile([P, NT * GD], _bf16, tag="wk")
            wv_s = pw.tile([P, NT * GD], _bf16, tag="wv")
            wo_s = pw.tile([P, GH * D], _bf16, tag="wo")
            cs_s = pcon.tile([P, S], _f32, tag="cs")
            ss_s = pcon.tile([P, S], _f32, tag="ss")
            ones_s = pcon.tile([P, P], _bf16, tag="ones")
            if causal:
                tri_s = pcon.tile([P, P], _bf16, tag="tri")
                nc.sync.dma_start(tri_s[:], tri[:])
            qt_s = pqkv.tile([P, GH * S], _bf16, tag="qt")
            kt_s = pqkv.tile([P, GH * S], _bf16, tag="kt")
            v_s = pqkv.tile([P, NT * GD], _bf16, tag="v")

            nc.sync.dma_start(cs_s[:], cs[:])
            nc.sync.dma_start(ss_s[:], ss[:])
            nc.sync.dma_start(ones_s[:], ones[:])
            for t in range(NT):
                nc.sync.dma_start(wq_s[:, t * GD:(t + 1) * GD], wq[t * P:(t + 1) * P, :])
            for t in range(NT):
                nc.sync.dma_start(wk_s[:, t * GD:(t + 1) * GD], wk[t * P:(t + 1) * P, :])
                nc.sync.dma_start(wv_s[:, t * GD:(t + 1) * GD], wv[t * P:(t + 1) * P, :])
            for dt in range(GH):
                nc.sync.dma_start(wo_s[:, dt * D:(dt + 1) * D], wo[dt * P:(dt + 1) * P, :])

            # ---- Phase P: Q^T/K^T (RoPE fused) and V projections -> SBUF
            for u in range(NU):
                su = slice(u * 512, (u + 1) * 512)
                xu = px.tile([P, NT * GD], _bf16, tag="xu")
                for t in range(NT):
                    nc.sync.dma_start(
                        xu[:, t * GD:(t + 1) * GD],
                        xT[t * P:(t + 1) * P, u * 512:(u + 1) * 512])
                for (w_s, dst) in ((wq_s, qt_s), (wk_s, kt_s)):
                    for dt in range(GH):
                        pq = psB.tile([P, 512], _f32, tag="ps")
                        for t in range(NT):
                            nc.tensor.matmul(
                                pq[:],
                                w_s[:, t * GD + dt * P: t * GD + dt * P + P],
                                xu[:, t * GD:(t + 1) * GD],
                                start=(t == 0), stop=(t == NT - 1))
                        t1 = ptmp.tile([P, 512], _f32, tag="t1")
                        t2 = ptmp.tile([P, 512], _f32, tag="t2")
                        nc.vector.tensor_mul(t1[:], pq[:], cs_s[:, su])
                        nc.vector.tensor_mul(t2[0:64, :], pq[64:P, :], ss_s[0:64, su])
                        nc.vector.tensor_mul(t2[64:P, :], pq[0:64, :], ss_s[64:P, su])
                        nc.vector.tensor_add(
                            dst[:, dt * S + u * 512: dt * S + (u + 1) * 512],
                            t1[:], t2[:])
                for st in range(4):
                    g = 4 * u + st
                    pv = psB.tile([P, GD], _f32, tag="ps")
                    for t in range(NT):
                        nc.tensor.matmul(
                            pv[:],
                            xu[:, t * GD + st * P: t * GD + st * P + P],
                            wv_s[:, t * GD:(t + 1) * GD],
                            start=(t == 0), stop=(t == NT - 1))
                    nc.scalar.copy(v_s[:, g * GD:(g + 1) * GD], pv[:])

            # ---- Phases A (attention) + W (output projection), staggered
            def attn(u, h):
                n_sk = 4 * (u + 1) if causal else NT
                if not causal:
                    mu = mus[u]
                psa = psAB.tile([P, 512], _f32, tag="psa")
                psd = psAB.tile([P, 512], _f32, tag="psd")
                pts = [None] * n_sk
                # denominator: per-tile (sliced) ones-matmuls into the psd chain
                ngrp = n_sk
                grp = {}      # last tile t -> (group idx, mm_lo, moving tile)
                gsums = {}

                def consume(t):
                    pt, lo = pts[t]
                    nc.tensor.matmul(psa[:, lo:512],
                                     v_s[:, t * GD + h * P: t * GD + (h + 1) * P],
                                     pt[:, lo:512],
                                     start=(t == 0), stop=(t == n_sk - 1))
                    if t in grp:
                        gi, mlo, stile = grp[t]
                        nc.tensor.matmul(psd[:, mlo:512], ones_s[:],
                                         stile[:, mlo:512],
                                         start=(gi == 0), stop=(gi == ngrp - 1))

                for t in range(n_sk):
                    d = t - 4 * u if causal else -1
                    lo = P * d if d > 0 else 0
                    pss = psB.tile([P, 512], _f32, tag="ps")
                    nc.tensor.matmul(
                        pss[:, lo:512],
                        kt_s[:, h * S + t * P: h * S + (t + 1) * P],
                        qt_s[:, h * S + u * 512 + lo: h * S + (u + 1) * 512],
                        start=True, stop=True)
                    if not causal:
                        nc.vector.tensor_add(
                            pss[:], pss[:], mu[:, t * 512:(t + 1) * 512])
                    pt = ppt.tile([P, 512], _bf16, tag="pt")
                    nc.scalar.activation(pt[:, lo:512], pss[:, lo:512], Exp)
                    if causal and d >= 0:
                        nc.gpsimd.tensor_mul(
                            pt[:, lo:lo + P], pt[:, lo:lo + P], tri_s[:])
                    pts[t] = (pt, lo)
                    grp[t] = (t, lo, pt)
                    if t >= depth:
                        consume(t - depth)
                for t in range(max(0, n_sk - depth), n_sk):
                    consume(t)
                rec = prec.tile([P, 512], _f32, tag="rec")
                nc.vector.reciprocal(rec[:], psd[:])
                nc.vector.tensor_mul(
                    aots[u][:, h * 512:(h + 1) * 512], psa[:], rec[:])

            def wproj(u, ots):
                for ot in ots:
                    po2 = psB.tile([P, 512], _f32, tag="ps")
                    for dt in range(GH):
                        nc.tensor.matmul(
                            po2[:],
                            wo_s[:, dt * D + ot * P: dt * D + (ot + 1) * P],
                            aots[u][:, dt * 512:(dt + 1) * 512],
                            start=(dt == 0), stop=(dt == GH - 1))
                    so = pso.tile([P, 512], _bf16, tag="so")
                    if ot % 2 == 0:
                        nc.vector.tensor_copy(so[:], po2[:])
                    else:
                        nc.scalar.copy(so[:], po2[:])
                    nc.sync.dma_start(
                        outT[ot * P:(ot + 1) * P, u * 512:(u + 1) * 512], so[:])

            aots = {}
            mus = {}
            for u in range(NU):
                aots[u] = paot.tile([P, GH * 512], _bf16, tag="aot", name="aot")
                if not causal:
                    mus[u] = pmu.tile([P, NT * 512], _bf16, tag="mu", name="mu")
                    for t in range(NT):
                        nc.sync.dma_start(
                            mus[u][:, t * 512:(t + 1) * 512],
                            maskf[t * P:(t + 1) * P, u * 512:(u + 1) * 512])
                attn(u, 0)
                if u > 0:
                    wproj(u - 1, range(NT))
                for h in range(1, GH):
                    attn(u, h)
            wproj(NU - 1, range(NT))
    nc.compile()
    return nc


class _Runner:
    """Persistent PJRT executable for one compiled Bass module (SPMD over 8 cores)."""

    def __init__(self, nc, n_cores):
        import jax
        from jax.sharding import Mesh, PartitionSpec
        from jax.experimental.shard_map import shard_map
        from concourse.bass2jax import (
            _bass_exec_p, install_neuronx_cc_hook, partition_id_tensor)

        install_neuronx_cc_hook()
        self.jax = jax
        self.n_cores = n_cores
        partition_name = nc.partition_id_tensor.name if nc.partition_id_tensor else None
        in_names, out_names, out_avals = [], [], []
        for alloc in nc.m.functions[0].allocations:
            if not isinstance(alloc, mybir.MemoryLocationSet):
                continue
            name = alloc.memorylocations[0].name
            if alloc.kind == "ExternalInput":
                if name != partition_name:
                    in_names.append(name)
            elif alloc.kind == "ExternalOutput":
                out_names.append(name)
                out_avals.append(jax.core.ShapedArray(
                    tuple(alloc.tensor_shape), mybir.dt.np(alloc.dtype)))
        self.in_names, self.out_names, self.out_avals = in_names, out_names, out_avals
        n_params, n_outs = len(in_names), len(out_avals)
        all_in = list(in_names) + list(out_names)
        if partition_name is not None:
            all_in.append(partition_name)

        def _body(*args):
            operands = list(args)
            if partition_name is not None:
                operands.append(partition_id_tensor())
            return tuple(_bass_exec_p.bind(
                *operands,
                out_avals=tuple(out_avals), in_names=tuple(all_in),
                out_names=tuple(out_names), lowering_input_output_aliases=(),
                sim_require_finite=True, sim_require_nnan=True, nc=nc))

        devices = jax.devices()[:n_cores]
        mesh = Mesh(np.asarray(devices), ("core",))
        self.sharding = jax.sharding.NamedSharding(mesh, PartitionSpec("core"))
        self.fn = jax.jit(
            shard_map(_body, mesh=mesh,
                      in_specs=(PartitionSpec("core"),) * (n_params + n_outs),
                      out_specs=(PartitionSpec("core"),) * n_outs,
                      check_rep=False),
            keep_unused=True)
        self._dev_args = None

    def put_inputs(self, in_maps):
        jax = self.jax
        concat_in = [
            np.concatenate([np.asarray(in_maps[c][n]) for c in range(self.n_cores)], axis=0)
            for n in self.in_names]
        concat_zeros = [
            np.zeros((self.n_cores * a.shape[0], *a.shape[1:]), a.dtype)
            for a in self.out_avals]
        self._dev_args = [
            jax.device_put(v, self.sharding) for v in concat_in + concat_zeros]
        for a in self._dev_args:
            a.block_until_ready()

    def execute(self):
        return self.fn(*self._dev_args)

    def run(self, in_maps):
        last_err = None
        for attempt in range(3):
            try:
                self.put_inputs(in_maps)
                outs = self.execute()
                self.jax.block_until_ready(outs)
                return [
                    {n: np.asarray(outs[i]).reshape(
                        self.n_cores, *self.out_avals[i].shape)[c]
                     for i, n in enumerate(self.out_names)}
                    for c in range(self.n_cores)]
            except Exception as e:  # transient NRT faults: retry
                last_err = e
                import time
                time.sleep(2.0 * (attempt + 1))
        raise last_err


def _get_runner(causal: bool):
    if causal not in _cache:
        _cache[causal] = _Runner(_build(causal), NCORES)
    return _cache[causal]


def _host_prep(x, mask, Wq, Wk, Wv, Wo, causal):
    scale = np.float32(1.0) / np.sqrt(np.float32(HD))
    perm = np.concatenate(
        [np.concatenate([np.arange(0, HD, 2), np.arange(1, HD, 2)]) + HD * hh
         for hh in range(GH)])
    inv = (np.float32(1.0) / np.power(
        np.float32(10000.0),
        np.arange(0, HD, 2).astype(np.float32) / np.float32(HD))).astype(np.float32)
    ang = np.arange(S, dtype=np.float32)[:, None] * inv[None, :]
    cos_t = np.cos(ang).T.astype(np.float32)
    sin_t = np.sin(ang).T.astype(np.float32)
    cs_host = np.ascontiguousarray(np.concatenate([cos_t, cos_t], axis=0))
    ss_host = np.ascontiguousarray(np.concatenate([-sin_t, sin_t], axis=0))
    ones_host = np.ones((P, P), _bf)
    if causal:
        # keep-mask: tri[r, j] = 1 where key r <= query j (within block), else 0
        tri_host = np.triu(np.ones((P, P), np.float32)).astype(_bf)
    else:
        maskT = np.ascontiguousarray(mask.T).astype(_bf)
    xTs = [np.ascontiguousarray(x[b].T).astype(_bf) for b in range(B)]
    in_maps = []
    for c in range(NCORES):
        b, g = c // G, c % G
        rows = slice(g * GD, (g + 1) * GD)
        m = {
            "xT": xTs[b],
            "wq": np.ascontiguousarray(Wq[rows].T[:, perm] * scale).astype(_bf),
            "wk": np.ascontiguousarray(Wk[rows].T[:, perm]).astype(_bf),
            "wv": np.ascontiguousarray(Wv[rows].T).astype(_bf),
            "wo": np.ascontiguousarray(Wo[:, rows].T).astype(_bf),
            "cs": cs_host,
            "ss": ss_host,
            "ones": ones_host,
        }
        if causal:
            m["tri"] = tri_host
        else:
            m["maskf"] = maskT
        in_maps.append(m)
    return in_maps


def kernel(x, mask, Wq, Wk, Wv, Wo):
    x = np.asarray(x, dtype=np.float32)
    mask = np.asarray(mask, dtype=np.float32)
    Wq = np.asarray(Wq, dtype=np.float32)
    Wk = np.asarray(Wk, dtype=np.float32)
    Wv = np.asarray(Wv, dtype=np.float32)
    Wo = np.asarray(Wo, dtype=np.float32)
    expected_mask = np.triu(np.full((S, S), -1e9, dtype=np.float32), k=1)
    causal = bool(np.array_equal(mask, expected_mask))
    runner = _get_runner(causal)
    in_maps = _host_prep(x, mask, Wq, Wk, Wv, Wo, causal)
    results = runner.run(in_maps)
    out = np.empty((B, S, D), np.float32)
    for b in range(B):
        acc = results[b * G]["outT"].astype(np.float32)
        for g in range(1, G):
            acc += results[b * G + g]["outT"].astype(np.float32)
        out[b] = acc.T
    return out


# revision 16
# speedup vs baseline: 1.4098x; 1.0386x over previous
"""Multi-head causal attention with RoPE on 8 TRN2 NeuronCores.

Sharding: 8 cores = 2 batches x 4 head-groups (4 heads each).
Per-core Bass kernel computes the group-partial output transposed;
host sums group partials and transposes back.

All matmul operands are bf16 (same PE throughput as fp32r, half the
SBUF/DMA footprint), accumulation in fp32 PSUM. Q^T/K^T/V stay
resident in SBUF (no DRAM scratch round-trip). Causal masking uses
a single triangular 128x128 tile: within each diagonal 128x512 score
tile only one 128-col block is partially masked; fully-masked columns
are skipped by slicing the matmul moving dim.
"""

import numpy as np
import ml_dtypes

import concourse.bass as bass  # noqa: F401
import concourse.tile as tile
from concourse import bacc, mybir

B, S, D, H, HD = 2, 2048, 2048, 16, 128
NCORES = 8
G = 4            # head groups
GH = 4           # heads per group
GD = GH * HD     # 512 dims per group
P = 128
NU = S // 512    # 4 query slices
NT = S // P      # 16 key tiles

_f32 = mybir.dt.float32
_bf16 = mybir.dt.bfloat16
_bf = ml_dtypes.bfloat16

_cache = {}


def _build(causal: bool, reps: int = 1, depth: int = 5):
    nc = bacc.Bacc("TRN2", target_bir_lowering=False, debug=False)
    xT = nc.dram_tensor("xT", [D, S], _bf16, kind="ExternalInput").ap()
    wq = nc.dram_tensor("wq", [D, GD], _bf16, kind="ExternalInput").ap()
    wk = nc.dram_tensor("wk", [D, GD], _bf16, kind="ExternalInput").ap()
    wv = nc.dram_tensor("wv", [D, GD], _bf16, kind="ExternalInput").ap()
    wo = nc.dram_tensor("wo", [GD, D], _bf16, kind="ExternalInput").ap()
    cs = nc.dram_tensor("cs", [P, S], _f32, kind="ExternalInput").ap()
    ss = nc.dram_tensor("ss", [P, S], _f32, kind="ExternalInput").ap()
    ones = nc.dram_tensor("ones", [P, P], _bf16, kind="ExternalInput").ap()
    if causal:
        tri = nc.dram_tensor("tri", [P, P], _bf16, kind="ExternalInput").ap()
    else:
        maskf = nc.dram_tensor("maskf", [S, S], _bf16, kind="ExternalInput").ap()
    outT = nc.dram_tensor("outT", [D, S], _bf16, kind="ExternalOutput").ap()

    Exp = mybir.ActivationFunctionType.Exp

    with tile.TileContext(nc) as tc:
      with (
          tc.tile_pool(name="pw", bufs=1) as pw,
          tc.tile_pool(name="pcon", bufs=1) as pcon,
          tc.tile_pool(name="px", bufs=2 if causal else 1) as px,
          tc.tile_pool(name="pqkv", bufs=1) as pqkv,
          tc.tile_pool(name="paot", bufs=2) as paot,
          tc.tile_pool(name="ptmp", bufs=2) as ptmp,
          tc.tile_pool(name="ppt", bufs=8) as ppt,
          tc.tile_pool(name="pds", bufs=6) as pds,
          tc.tile_pool(name="prec", bufs=2) as prec,
          tc.tile_pool(name="pso", bufs=8) as pso,
          tc.tile_pool(name="pmu", bufs=1) as pmu,
          tc.tile_pool(name="psAB", bufs=2, space="PSUM") as psAB,
          tc.tile_pool(name="psB", bufs=4, space="PSUM") as psB,
      ):
        for _rep in range(reps):
            # ---- per-rep input tiles (pools persist; tags rotate buffers)
            wq_s = pw.tile([P, NT * GD], _bf16, tag="wq")
            wk_s = pw.tile([P, NT * GD], _bf16, tag="wk")
            wv_s = pw.tile([P, NT * GD], _bf16, tag="wv")
            wo_s = pw.tile([P, GH * D], _bf16, tag="wo")
            cs_s = pcon.tile([P, S], _f32, tag="cs")
            ss_s = pcon.tile([P, S], _f32, tag="ss")
            ones_s = pcon.tile([P, P], _bf16, tag="ones")
            if causal:
                tri_s = pcon.tile([P, P], _bf16, tag="tri")
                nc.sync.dma_start(tri_s[:], tri[:])
            qt_s = pqkv.tile([P, GH * S], _bf16, tag="qt")
            kt_s = pqkv.tile([P, GH * S], _bf16, tag="kt")
            v_s = pqkv.tile([P, NT * GD], _bf16, tag="v")

            nc.sync.dma_start(cs_s[:], cs[:])
            nc.sync.dma_start(ss_s[:], ss[:])
            nc.sync.dma_start(ones_s[:], ones[:])
            for (w_s, w_d) in ((wq_s, wq), (wk_s, wk), (wv_s, wv)):
                nc.sync.dma_start(
                    w_s[:].rearrange("p (t j) -> p t j", t=NT),
                    w_d.rearrange("(t p) j -> p t j", t=NT))
            nc.sync.dma_start(
                wo_s[:].rearrange("p (d j) -> p d j", d=GH),
                wo.rearrange("(d p) j -> p d j", d=GH))

            # ---- Phase P: Q^T/K^T (RoPE fused) and V projections -> SBUF
            for u in range(NU):
                su = slice(u * 512, (u + 1) * 512)
                xu = px.tile([P, NT * GD], _bf16, tag="xu")
                nc.sync.dma_start(
                    xu[:].rearrange("p (t j) -> p t j", t=NT),
                    xT[:, u * 512:(u + 1) * 512].rearrange("(t p) j -> p t j", t=NT))
                for (w_s, dst) in ((wq_s, qt_s), (wk_s, kt_s)):
                    for dt in range(GH):
                        pq = psB.tile([P, 512], _f32, tag="ps")
                        for t in range(NT):
                            nc.tensor.matmul(
                                pq[:],
                                w_s[:, t * GD + dt * P: t * GD + dt * P + P],
                                xu[:, t * GD:(t + 1) * GD],
                                start=(t == 0), stop=(t == NT - 1))
                        t1 = ptmp.tile([P, 512], _f32, tag="t1")
                        t2 = ptmp.tile([P, 512], _f32, tag="t2")
                        nc.vector.tensor_mul(t1[:], pq[:], cs_s[:, su])
                        nc.vector.tensor_mul(t2[0:64, :], pq[64:P, :], ss_s[0:64, su])
                        nc.vector.tensor_mul(t2[64:P, :], pq[0:64, :], ss_s[64:P, su])
                        nc.vector.tensor_add(
                            dst[:, dt * S + u * 512: dt * S + (u + 1) * 512],
                            t1[:], t2[:])
                for st in range(4):
                    g = 4 * u + st
                    pv = psB.tile([P, GD], _f32, tag="ps")
                    for t in range(NT):
                        nc.tensor.matmul(
                            pv[:],
                            xu[:, t * GD + st * P: t * GD + st * P + P],
                            wv_s[:, t * GD:(t + 1) * GD],
                            start=(t == 0), stop=(t == NT - 1))
                    nc.scalar.copy(v_s[:, g * GD:(g + 1) * GD], pv[:])

            # ---- Phases A (attention) + W (output projection), staggered
            def attn(u, h):
                n_sk = 4 * (u + 1) if causal else NT
                if not causal:
                    mu = mus[u]
                psa = psAB.tile([P, 512], _f32, tag="psa")
                psd = psAB.tile([P, 512], _f32, tag="psd")
                pts = [None] * n_sk
                # denominator: per-tile (sliced) ones-matmuls, except off-diag
                # quads tree-summed on DVE for deep chains (u >= 2)
                useq = causal and u >= 2
                nq = (4 * u) // 4 if useq else 0
                ngrp = (nq + 4 + (0 if useq else 4 * u)) if causal else n_sk
                grp = {}      # last tile t -> (group idx, mm_lo, moving tile)
                gsums = {}

                def consume(t):
                    pt, lo = pts[t]
                    nc.tensor.matmul(psa[:, lo:512],
                                     v_s[:, t * GD + h * P: t * GD + (h + 1) * P],
                                     pt[:, lo:512],
                                     start=(t == 0), stop=(t == n_sk - 1))
                    if t in grp:
                        gi, mlo, stile = grp[t]
                        nc.tensor.matmul(psd[:, mlo:512], ones_s[:],
                                         stile[:, mlo:512],
                                         start=(gi == 0), stop=(gi == ngrp - 1))

                for t in range(n_sk):
                    d = t - 4 * u if causal else -1
                    lo = P * d if d > 0 else 0
                    pss = psB.tile([P, 512], _f32, tag="ps")
                    nc.tensor.matmul(
                        pss[:, lo:512],
                        kt_s[:, h * S + t * P: h * S + (t + 1) * P],
                        qt_s[:, h * S + u * 512 + lo: h * S + (u + 1) * 512],
                        start=True, stop=True)
                    if not causal:
                        nc.vector.tensor_add(
                            pss[:], pss[:], mu[:, t * 512:(t + 1) * 512])
                    pt = ppt.tile([P, 512], _bf16, tag="pt")
                    nc.scalar.activation(pt[:, lo:512], pss[:, lo:512], Exp)
                    if causal and d >= 0:
                        nc.gpsimd.tensor_mul(
                            pt[:, lo:lo + P], pt[:, lo:lo + P], tri_s[:])
                    pts[t] = (pt, lo)
                    if useq and d < 0:
                        if t % 4 == 1:
                            ds = pds.tile([P, 512], _bf16, tag="ds", name="ds")
                            gsums[t // 4] = ds
                            nc.vector.tensor_add(ds[:], pts[t - 1][0][:], pt[:])
                        elif t % 4 == 3:
                            ds2 = pds.tile([P, 512], _bf16, tag="ds", name="ds2")
                            nc.vector.tensor_add(ds2[:], pts[t - 1][0][:], pt[:])
                            ds = gsums[t // 4]
                            nc.vector.tensor_add(ds[:], ds[:], ds2[:])
                            grp[t] = (t // 4, 0, ds)
                    elif useq:
                        grp[t] = (nq + d, lo, pt)
                    else:
                        grp[t] = (t, lo, pt)
                    if t >= depth:
                        consume(t - depth)
                for t in range(max(0, n_sk - depth), n_sk):
                    consume(t)
                rec = prec.tile([P, 512], _f32, tag="rec")
                nc.vector.reciprocal(rec[:], psd[:])
                nc.vector.tensor_mul(
                    aots[u][:, h * 512:(h + 1) * 512], psa[:], rec[:])

            def wproj(u, ots):
                for ot in ots:
                    po2 = psB.tile([P, 512], _f32, tag="ps")
                    for dt in range(GH):
                        nc.tensor.matmul(
                            po2[:],
                            wo_s[:, dt * D + ot * P: dt * D + (ot + 1) * P],
                            aots[u][:, dt * 512:(dt + 1) * 512],
                            start=(dt == 0), stop=(dt == GH - 1))
                    so = pso.tile([P, 512], _bf16, tag="so")
                    if ot % 2 == 0:
                        nc.vector.tensor_copy(so[:], po2[:])
                    else:
                        nc.scalar.copy(so[:], po2[:])
                    nc.sync.dma_start(
                        outT[ot * P:(ot + 1) * P, u * 512:(u + 1) * 512], so[:])

            aots = {}
            mus = {}
            for u in range(NU):
                aots[u] = paot.tile([P, GH * 512], _bf16, tag="aot", name="aot")
                if not causal:
                    mus[u] = pmu.tile([P, NT * 512], _bf16, tag="mu", name="mu")
                    nc.sync.dma_start(
                        mus[u][:].rearrange("p (t j) -> p t j", t=NT),
                        maskf[:, u * 512:(u + 1) * 512].rearrange(
                            "(t p) j -> p t j", t=NT))
                attn(u, 0)
                if u > 0:
                    wproj(u - 1, range(NT))
                for h in range(1, GH):
                    attn(u, h)
            wproj(NU - 1, range(NT))
    nc.compile()
    return nc


class _Runner:
    """Persistent PJRT executable for one compiled Bass module (SPMD over 8 cores)."""

    def __init__(self, nc, n_cores):
        import jax
        from jax.sharding import Mesh, PartitionSpec
        from jax.experimental.shard_map import shard_map
        from concourse.bass2jax import (
            _bass_exec_p, install_neuronx_cc_hook, partition_id_tensor)

        install_neuronx_cc_hook()
        self.jax = jax
        self.n_cores = n_cores
        partition_name = nc.partition_id_tensor.name if nc.partition_id_tensor else None
        in_names, out_names, out_avals = [], [], []
        for alloc in nc.m.functions[0].allocations:
            if not isinstance(alloc, mybir.MemoryLocationSet):
                continue
            name = alloc.memorylocations[0].name
            if alloc.kind == "ExternalInput":
                if name != partition_name:
                    in_names.append(name)
            elif alloc.kind == "ExternalOutput":
                out_names.append(name)
                out_avals.append(jax.core.ShapedArray(
                    tuple(alloc.tensor_shape), mybir.dt.np(alloc.dtype)))
        self.in_names, self.out_names, self.out_avals = in_names, out_names, out_avals
        n_params, n_outs = len(in_names), len(out_avals)
        all_in = list(in_names) + list(out_names)
        if partition_name is not None:
            all_in.append(partition_name)

        def _body(*args):
            operands = list(args)
            if partition_name is not None:
                operands.append(partition_id_tensor())
            return tuple(_bass_exec_p.bind(
                *operands,
                out_avals=tuple(out_avals), in_names=tuple(all_in),
                out_names=tuple(out_names), lowering_input_output_aliases=(),
                sim_require_finite=True, sim_require_nnan=True, nc=nc))

        devices = jax.devices()[:n_cores]
        mesh = Mesh(np.asarray(devices), ("core",))
        self.sharding = jax.sharding.NamedSharding(mesh, PartitionSpec("core"))
        self.fn = jax.jit(
            shard_map(_body, mesh=mesh,
                      in_specs=(PartitionSpec("core"),) * (n_params + n_outs),
                      out_specs=(PartitionSpec("core"),) * n_outs,
                      check_rep=False),
            keep_unused=True)
        self._dev_args = None

    def put_inputs(self, in_maps):
        jax = self.jax
        concat_in = [
            np.concatenate([np.asarray(in_maps[c][n]) for c in range(self.n_cores)], axis=0)
            for n in self.in_names]
        concat_zeros = [
            np.zeros((self.n_cores * a.shape[0], *a.shape[1:]), a.dtype)
            for a in self.out_avals]
        self._dev_args = [
            jax.device_put(v, self.sharding) for v in concat_in + concat_zeros]
        for a in self._dev_args:
            a.block_until_ready()

    def execute(self):
        return self.fn(*self._dev_args)

    def run(self, in_maps):
        last_err = None
        for attempt in range(3):
            try:
                self.put_inputs(in_maps)
                outs = self.execute()
                self.jax.block_until_ready(outs)
                return [
                    {n: np.asarray(outs[i]).reshape(
                        self.n_cores, *self.out_avals[i].shape)[c]
                     for i, n in enumerate(self.out_names)}
                    for c in range(self.n_cores)]
            except Exception as e:  # transient NRT faults: retry
                last_err = e
                import time
                time.sleep(2.0 * (attempt + 1))
        raise last_err


def _get_runner(causal: bool):
    if causal not in _cache:
        _cache[causal] = _Runner(_build(causal), NCORES)
    return _cache[causal]


def _host_prep(x, mask, Wq, Wk, Wv, Wo, causal):
    scale = np.float32(1.0) / np.sqrt(np.float32(HD))
    perm = np.concatenate(
        [np.concatenate([np.arange(0, HD, 2), np.arange(1, HD, 2)]) + HD * hh
         for hh in range(GH)])
    inv = (np.float32(1.0) / np.power(
        np.float32(10000.0),
        np.arange(0, HD, 2).astype(np.float32) / np.float32(HD))).astype(np.float32)
    ang = np.arange(S, dtype=np.float32)[:, None] * inv[None, :]
    cos_t = np.cos(ang).T.astype(np.float32)
    sin_t = np.sin(ang).T.astype(np.float32)
    cs_host = np.ascontiguousarray(np.concatenate([cos_t, cos_t], axis=0))
    ss_host = np.ascontiguousarray(np.concatenate([-sin_t, sin_t], axis=0))
    ones_host = np.ones((P, P), _bf)
    if causal:
        # keep-mask: tri[r, j] = 1 where key r <= query j (within block), else 0
        tri_host = np.triu(np.ones((P, P), np.float32)).astype(_bf)
    else:
        maskT = np.ascontiguousarray(mask.T).astype(_bf)
    xTs = [np.ascontiguousarray(x[b].T).astype(_bf) for b in range(B)]
    in_maps = []
    for c in range(NCORES):
        b, g = c // G, c % G
        rows = slice(g * GD, (g + 1) * GD)
        m = {
            "xT": xTs[b],
            "wq": np.ascontiguousarray(Wq[rows].T[:, perm] * scale).astype(_bf),
            "wk": np.ascontiguousarray(Wk[rows].T[:, perm]).astype(_bf),
            "wv": np.ascontiguousarray(Wv[rows].T).astype(_bf),
            "wo": np.ascontiguousarray(Wo[:, rows].T).astype(_bf),
            "cs": cs_host,
            "ss": ss_host,
            "ones": ones_host,
        }
        if causal:
            m["tri"] = tri_host
        else:
            m["maskf"] = maskT
        in_maps.append(m)
    return in_maps


def kernel(x, mask, Wq, Wk, Wv, Wo):
    x = np.asarray(x, dtype=np.float32)
    mask = np.asarray(mask, dtype=np.float32)
    Wq = np.asarray(Wq, dtype=np.float32)
    Wk = np.asarray(Wk, dtype=np.float32)
    Wv = np.asarray(Wv, dtype=np.float32)
    Wo = np.asarray(Wo, dtype=np.float32)
    expected_mask = np.triu(np.full((S, S), -1e9, dtype=np.float32), k=1)
    causal = bool(np.array_equal(mask, expected_mask))
    runner = _get_runner(causal)
    in_maps = _host_prep(x, mask, Wq, Wk, Wv, Wo, causal)
    results = runner.run(in_maps)
    out = np.empty((B, S, D), np.float32)
    for b in range(B):
        acc = results[b * G]["outT"].astype(np.float32)
        for g in range(1, G):
            acc += results[b * G + g]["outT"].astype(np.float32)
        out[b] = acc.T
    return out


# revision 19
# speedup vs baseline: 1.4845x; 1.0530x over previous
"""Multi-head causal attention with RoPE on 8 TRN2 NeuronCores.

Sharding: 8 cores = 2 batches x 4 head-groups (4 heads each).
Per-core Bass kernel computes the group-partial output transposed;
host sums group partials and transposes back.

All matmul operands are bf16 (same PE throughput as fp32r, half the
SBUF/DMA footprint), accumulation in fp32 PSUM. Q^T/K^T/V stay
resident in SBUF (no DRAM scratch round-trip). Causal masking uses
a single triangular 128x128 tile: within each diagonal 128x512 score
tile only one 128-col block is partially masked; fully-masked columns
are skipped by slicing the matmul moving dim.
"""

import numpy as np
import ml_dtypes

import concourse.bass as bass  # noqa: F401
import concourse.tile as tile
from concourse import bacc, mybir

B, S, D, H, HD = 2, 2048, 2048, 16, 128
NCORES = 8
G = 4            # head groups
GH = 4           # heads per group
GD = GH * HD     # 512 dims per group
P = 128
NU = S // 512    # 4 query slices
NT = S // P      # 16 key tiles

_f32 = mybir.dt.float32
_bf16 = mybir.dt.bfloat16
_bf = ml_dtypes.bfloat16

_cache = {}


def _build(causal: bool, reps: int = 1, depth: int = 7):
    nc = bacc.Bacc("TRN2", target_bir_lowering=False, debug=False)
    xT = nc.dram_tensor("xT", [D, S], _bf16, kind="ExternalInput").ap()
    wq = nc.dram_tensor("wq", [D, GD], _bf16, kind="ExternalInput").ap()
    wk = nc.dram_tensor("wk", [D, GD], _bf16, kind="ExternalInput").ap()
    wv = nc.dram_tensor("wv", [D, GD], _bf16, kind="ExternalInput").ap()
    wo = nc.dram_tensor("wo", [GD, D], _bf16, kind="ExternalInput").ap()
    cs = nc.dram_tensor("cs", [P, S], _f32, kind="ExternalInput").ap()
    ss = nc.dram_tensor("ss", [P, S], _f32, kind="ExternalInput").ap()
    ones = nc.dram_tensor("ones", [P, P], _bf16, kind="ExternalInput").ap()
    if causal:
        tri = nc.dram_tensor("tri", [P, P], _bf16, kind="ExternalInput").ap()
    else:
        maskf = nc.dram_tensor("maskf", [S, S], _bf16, kind="ExternalInput").ap()
    outT = nc.dram_tensor("outT", [D, S], _bf16, kind="ExternalOutput").ap()

    Exp = mybir.ActivationFunctionType.Exp

    with tile.TileContext(nc) as tc:
      with (
          tc.tile_pool(name="pw", bufs=1) as pw,
          tc.tile_pool(name="pcon", bufs=1) as pcon,
          tc.tile_pool(name="px", bufs=2 if causal else 1) as px,
          tc.tile_pool(name="pqkv", bufs=1) as pqkv,
          tc.tile_pool(name="paot", bufs=2) as paot,
          tc.tile_pool(name="ptmp", bufs=2) as ptmp,
          tc.tile_pool(name="ppt", bufs=8) as ppt,
          tc.tile_pool(name="pds", bufs=6) as pds,
          tc.tile_pool(name="prec", bufs=2) as prec,
          tc.tile_pool(name="pso", bufs=8) as pso,
          tc.tile_pool(name="pmu", bufs=1) as pmu,
          tc.tile_pool(name="psAB", bufs=2, space="PSUM") as psAB,
          tc.tile_pool(name="psB", bufs=4, space="PSUM") as psB,
      ):
        for _rep in range(reps):
            # ---- per-rep input tiles (pools persist; tags rotate buffers)
            wq_s = pw.tile([P, NT * GD], _bf16, tag="wq")
            wk_s = pw.tile([P, NT * GD], _bf16, tag="wk")
            wv_s = pw.tile([P, NT * GD], _bf16, tag="wv")
            wo_s = pw.tile([P, GH * D], _bf16, tag="wo")
            cs_s = pcon.tile([P, S], _f32, tag="cs")
            ss_s = pcon.tile([P, S], _f32, tag="ss")
            ones_s = pcon.tile([P, P], _bf16, tag="ones")
            if causal:
                tri_s = pcon.tile([P, P], _bf16, tag="tri")
                nc.sync.dma_start(tri_s[:], tri[:])
            qt_s = pqkv.tile([P, GH * S], _bf16, tag="qt")
            kt_s = pqkv.tile([P, GH * S], _bf16, tag="kt")
            v_s = pqkv.tile([P, NT * GD], _bf16, tag="v")

            nc.sync.dma_start(cs_s[:], cs[:])
            nc.sync.dma_start(ss_s[:], ss[:])
            nc.sync.dma_start(ones_s[:], ones[:])
            for (w_s, w_d) in ((wq_s, wq), (wk_s, wk), (wv_s, wv)):
                nc.sync.dma_start(
                    w_s[:].rearrange("p (t j) -> p t j", t=NT),
                    w_d.rearrange("(t p) j -> p t j", t=NT))
            nc.sync.dma_start(
                wo_s[:].rearrange("p (d j) -> p d j", d=GH),
                wo.rearrange("(d p) j -> p d j", d=GH))

            # ---- Phase P: Q^T/K^T (RoPE fused) and V projections -> SBUF
            for u in range(NU):
                su = slice(u * 512, (u + 1) * 512)
                xu = px.tile([P, NT * GD], _bf16, tag="xu")
                nc.sync.dma_start(
                    xu[:].rearrange("p (t j) -> p t j", t=NT),
                    xT[:, u * 512:(u + 1) * 512].rearrange("(t p) j -> p t j", t=NT))
                for (w_s, dst) in ((wq_s, qt_s), (wk_s, kt_s)):
                    for dt in range(GH):
                        pq = psB.tile([P, 512], _f32, tag="ps")
                        for t in range(NT):
                            nc.tensor.matmul(
                                pq[:],
                                w_s[:, t * GD + dt * P: t * GD + dt * P + P],
                                xu[:, t * GD:(t + 1) * GD],
                                start=(t == 0), stop=(t == NT - 1))
                        t1 = ptmp.tile([P, 512], _f32, tag="t1")
                        t2 = ptmp.tile([P, 512], _f32, tag="t2")
                        nc.vector.tensor_mul(t1[:], pq[:], cs_s[:, su])
                        nc.vector.tensor_mul(t2[0:64, :], pq[64:P, :], ss_s[0:64, su])
                        nc.vector.tensor_mul(t2[64:P, :], pq[0:64, :], ss_s[64:P, su])
                        nc.vector.tensor_add(
                            dst[:, dt * S + u * 512: dt * S + (u + 1) * 512],
                            t1[:], t2[:])
                for st in range(4):
                    g = 4 * u + st
                    pv = psB.tile([P, GD], _f32, tag="ps")
                    for t in range(NT):
                        nc.tensor.matmul(
                            pv[:],
                            xu[:, t * GD + st * P: t * GD + st * P + P],
                            wv_s[:, t * GD:(t + 1) * GD],
                            start=(t == 0), stop=(t == NT - 1))
                    nc.scalar.copy(v_s[:, g * GD:(g + 1) * GD], pv[:])

            # ---- Phases A (attention) + W (output projection), staggered
            def attn(u, h):
                n_sk = 4 * (u + 1) if causal else NT
                if not causal:
                    mu = mus[u]
                psa = psAB.tile([P, 512], _f32, tag="psa")
                psd = psAB.tile([P, 512], _f32, tag="psd")
                pts = [None] * n_sk
                # denominator: per-tile (sliced) ones-matmuls, except off-diag
                # quads tree-summed on DVE for deep chains (u >= 2)
                useq = False and causal
                nq = (4 * u) // 4 if useq else 0
                ngrp = (nq + 4 + (0 if useq else 4 * u)) if causal else n_sk
                grp = {}      # last tile t -> (group idx, mm_lo, moving tile)
                gsums = {}

                def consume(t):
                    pt, lo = pts[t]
                    nc.tensor.matmul(psa[:, lo:512],
                                     v_s[:, t * GD + h * P: t * GD + (h + 1) * P],
                                     pt[:, lo:512],
                                     start=(t == 0), stop=(t == n_sk - 1))
                    if t in grp:
                        gi, mlo, stile = grp[t]
                        nc.tensor.matmul(psd[:, mlo:512], ones_s[:],
                                         stile[:, mlo:512],
                                         start=(gi == 0), stop=(gi == ngrp - 1))

                for t in range(n_sk):
                    d = t - 4 * u if causal else -1
                    lo = P * d if d > 0 else 0
                    pss = psB.tile([P, 512], _f32, tag="ps")
                    nc.tensor.matmul(
                        pss[:, lo:512],
                        kt_s[:, h * S + t * P: h * S + (t + 1) * P],
                        qt_s[:, h * S + u * 512 + lo: h * S + (u + 1) * 512],
                        start=True, stop=True)
                    if not causal:
                        nc.vector.tensor_add(
                            pss[:], pss[:], mu[:, t * 512:(t + 1) * 512])
                    pt = ppt.tile([P, 512], _bf16, tag="pt")
                    nc.scalar.activation(pt[:, lo:512], pss[:, lo:512], Exp)
                    if causal and d >= 0:
                        nc.gpsimd.tensor_mul(
                            pt[:, lo:lo + P], pt[:, lo:lo + P], tri_s[:])
                    pts[t] = (pt, lo)
                    if useq and d < 0:
                        if t % 4 == 1:
                            ds = pds.tile([P, 512], _bf16, tag="ds", name="ds")
                            gsums[t // 4] = ds
                            nc.vector.tensor_add(ds[:], pts[t - 1][0][:], pt[:])
                        elif t % 4 == 3:
                            ds2 = pds.tile([P, 512], _bf16, tag="ds", name="ds2")
                            nc.vector.tensor_add(ds2[:], pts[t - 1][0][:], pt[:])
                            ds = gsums[t // 4]
                            nc.vector.tensor_add(ds[:], ds[:], ds2[:])
                            grp[t] = (t // 4, 0, ds)
                    elif useq:
                        grp[t] = (nq + d, lo, pt)
                    else:
                        grp[t] = (t, lo, pt)
                    if t >= depth:
                        consume(t - depth)
                for t in range(max(0, n_sk - depth), n_sk):
                    consume(t)
                rec = prec.tile([P, 512], _f32, tag="rec")
                nc.vector.reciprocal(rec[:], psd[:])
                nc.vector.tensor_mul(
                    aots[u][:, h * 512:(h + 1) * 512], psa[:], rec[:])

            def wproj(u, ots):
                for ot in ots:
                    if ot == NT - 2:
                        po2 = psAB.tile([P, 512], _f32, tag="psa", name="po2a")
                    elif ot == NT - 1:
                        po2 = psAB.tile([P, 512], _f32, tag="psd", name="po2d")
                    else:
                        po2 = psB.tile([P, 512], _f32, tag="ps")
                    for dt in range(GH):
                        nc.tensor.matmul(
                            po2[:],
                            wo_s[:, dt * D + ot * P: dt * D + (ot + 1) * P],
                            aots[u][:, dt * 512:(dt + 1) * 512],
                            start=(dt == 0), stop=(dt == GH - 1))
                    so = pso.tile([P, 512], _bf16, tag="so")
                    if ot % 2 == 0:
                        nc.vector.tensor_copy(so[:], po2[:])
                    else:
                        nc.scalar.copy(so[:], po2[:])
                    nc.sync.dma_start(
                        outT[ot * P:(ot + 1) * P, u * 512:(u + 1) * 512], so[:])

            aots = {}
            mus = {}
            for u in range(NU):
                aots[u] = paot.tile([P, GH * 512], _bf16, tag="aot", name="aot")
                if not causal:
                    mus[u] = pmu.tile([P, NT * 512], _bf16, tag="mu", name="mu")
                    nc.sync.dma_start(
                        mus[u][:].rearrange("p (t j) -> p t j", t=NT),
                        maskf[:, u * 512:(u + 1) * 512].rearrange(
                            "(t p) j -> p t j", t=NT))
                attn(u, 0)
                if u > 0:
                    wproj(u - 1, range(NT))
                for h in range(1, GH):
                    attn(u, h)
            wproj(NU - 1, range(NT))
    nc.compile()
    return nc


class _Runner:
    """Persistent PJRT executable for one compiled Bass module (SPMD over 8 cores)."""

    def __init__(self, nc, n_cores):
        import jax
        from jax.sharding import Mesh, PartitionSpec
        from jax.experimental.shard_map import shard_map
        from concourse.bass2jax import (
            _bass_exec_p, install_neuronx_cc_hook, partition_id_tensor)

        install_neuronx_cc_hook()
        self.jax = jax
        self.n_cores = n_cores
        partition_name = nc.partition_id_tensor.name if nc.partition_id_tensor else None
        in_names, out_names, out_avals = [], [], []
        for alloc in nc.m.functions[0].allocations:
            if not isinstance(alloc, mybir.MemoryLocationSet):
                continue
            name = alloc.memorylocations[0].name
            if alloc.kind == "ExternalInput":
                if name != partition_name:
                    in_names.append(name)
            elif alloc.kind == "ExternalOutput":
                out_names.append(name)
                out_avals.append(jax.core.ShapedArray(
                    tuple(alloc.tensor_shape), mybir.dt.np(alloc.dtype)))
        self.in_names, self.out_names, self.out_avals = in_names, out_names, out_avals
        n_params, n_outs = len(in_names), len(out_avals)
        all_in = list(in_names) + list(out_names)
        if partition_name is not None:
            all_in.append(partition_name)

        def _body(*args):
            operands = list(args)
            if partition_name is not None:
                operands.append(partition_id_tensor())
            return tuple(_bass_exec_p.bind(
                *operands,
                out_avals=tuple(out_avals), in_names=tuple(all_in),
                out_names=tuple(out_names), lowering_input_output_aliases=(),
                sim_require_finite=True, sim_require_nnan=True, nc=nc))

        devices = jax.devices()[:n_cores]
        mesh = Mesh(np.asarray(devices), ("core",))
        self.sharding = jax.sharding.NamedSharding(mesh, PartitionSpec("core"))
        self.fn = jax.jit(
            shard_map(_body, mesh=mesh,
                      in_specs=(PartitionSpec("core"),) * (n_params + n_outs),
                      out_specs=(PartitionSpec("core"),) * n_outs,
                      check_rep=False),
            keep_unused=True)
        self._dev_args = None

    def put_inputs(self, in_maps):
        jax = self.jax
        concat_in = [
            np.concatenate([np.asarray(in_maps[c][n]) for c in range(self.n_cores)], axis=0)
            for n in self.in_names]
        concat_zeros = [
            np.zeros((self.n_cores * a.shape[0], *a.shape[1:]), a.dtype)
            for a in self.out_avals]
        self._dev_args = [
            jax.device_put(v, self.sharding) for v in concat_in + concat_zeros]
        for a in self._dev_args:
            a.block_until_ready()

    def execute(self):
        return self.fn(*self._dev_args)

    def run(self, in_maps):
        last_err = None
        for attempt in range(3):
            try:
                self.put_inputs(in_maps)
                outs = self.execute()
                self.jax.block_until_ready(outs)
                return [
                    {n: np.asarray(outs[i]).reshape(
                        self.n_cores, *self.out_avals[i].shape)[c]
                     for i, n in enumerate(self.out_names)}
                    for c in range(self.n_cores)]
            except Exception as e:  # transient NRT faults: retry
                last_err = e
                import time
                time.sleep(2.0 * (attempt + 1))
        raise last_err


def _get_runner(causal: bool):
    if causal not in _cache:
        _cache[causal] = _Runner(_build(causal), NCORES)
    return _cache[causal]


def _host_prep(x, mask, Wq, Wk, Wv, Wo, causal):
    scale = np.float32(1.0) / np.sqrt(np.float32(HD))
    perm = np.concatenate(
        [np.concatenate([np.arange(0, HD, 2), np.arange(1, HD, 2)]) + HD * hh
         for hh in range(GH)])
    inv = (np.float32(1.0) / np.power(
        np.float32(10000.0),
        np.arange(0, HD, 2).astype(np.float32) / np.float32(HD))).astype(np.float32)
    ang = np.arange(S, dtype=np.float32)[:, None] * inv[None, :]
    cos_t = np.cos(ang).T.astype(np.float32)
    sin_t = np.sin(ang).T.astype(np.float32)
    cs_host = np.ascontiguousarray(np.concatenate([cos_t, cos_t], axis=0))
    ss_host = np.ascontiguousarray(np.concatenate([-sin_t, sin_t], axis=0))
    ones_host = np.ones((P, P), _bf)
    if causal:
        # keep-mask: tri[r, j] = 1 where key r <= query j (within block), else 0
        tri_host = np.triu(np.ones((P, P), np.float32)).astype(_bf)
    else:
        maskT = np.ascontiguousarray(mask.T).astype(_bf)
    xTs = [np.ascontiguousarray(x[b].T).astype(_bf) for b in range(B)]
    in_maps = []
    for c in range(NCORES):
        b, g = c // G, c % G
        rows = slice(g * GD, (g + 1) * GD)
        m = {
            "xT": xTs[b],
            "wq": np.ascontiguousarray(Wq[rows].T[:, perm] * scale).astype(_bf),
            "wk": np.ascontiguousarray(Wk[rows].T[:, perm]).astype(_bf),
            "wv": np.ascontiguousarray(Wv[rows].T).astype(_bf),
            "wo": np.ascontiguousarray(Wo[:, rows].T).astype(_bf),
            "cs": cs_host,
            "ss": ss_host,
            "ones": ones_host,
        }
        if causal:
            m["tri"] = tri_host
        else:
            m["maskf"] = maskT
        in_maps.append(m)
    return in_maps


def kernel(x, mask, Wq, Wk, Wv, Wo):
    x = np.asarray(x, dtype=np.float32)
    mask = np.asarray(mask, dtype=np.float32)
    Wq = np.asarray(Wq, dtype=np.float32)
    Wk = np.asarray(Wk, dtype=np.float32)
    Wv = np.asarray(Wv, dtype=np.float32)
    Wo = np.asarray(Wo, dtype=np.float32)
    expected_mask = np.triu(np.full((S, S), -1e9, dtype=np.float32), k=1)
    causal = bool(np.array_equal(mask, expected_mask))
    runner = _get_runner(causal)
    in_maps = _host_prep(x, mask, Wq, Wk, Wv, Wo, causal)
    results = runner.run(in_maps)
    out = np.empty((B, S, D), np.float32)
    for b in range(B):
        acc = results[b * G]["outT"].astype(np.float32)
        for g in range(1, G):
            acc += results[b * G + g]["outT"].astype(np.float32)
        out[b] = acc.T
    return out
